# revision 36
# baseline (speedup 1.0000x reference)
"""BestRQ VQ kernel for 8 TRN2 NeuronCores.

Data-parallel over batch: core b handles feats[b] (299 stacked rows).
codes = argmax_c <x_g, cb_cg>  (L2-norm of x is a positive per-row scalar and
the codebook rows are unit-norm, so normalization does not change the argmin).
quantized = codebook[codes] gathered via indirect DMA.
"""

import os
import sys

import numpy as np

sys.path.insert(0, "/opt/trn_rl_repo")

B, T, M = 8, 1200, 80
F, S = 7, 4
T2 = (T - F) // S + 1  # 299
D, G, Dg, C = 512, 8, 64, 8192
KIN = F * M  # 560
KT = 112  # KIN split into 5 tiles of 112
NKT = 5
NCORES = 8
ROW_CHUNKS = [(0, 128), (128, 128), (256, T2 - 256)]  # (start, nrows)
NCT = C // 512  # 16 c-tiles of 512

_CACHE = {}


def _build():
    import concourse.bass as bass
    import concourse.tile as tile
    from concourse import bacc, mybir
    from concourse.masks import make_identity

    f32 = mybir.dt.float32
    f32r = mybir.dt.float32r
    u32 = mybir.dt.uint32

    nc = bacc.Bacc(None, target_bir_lowering=False)

    feats = nc.declare_dram_parameter("feats", [T, M], f32, isOutput=False)
    proj = nc.declare_dram_parameter("proj", [KIN, D], f32, isOutput=False)
    cbt = nc.declare_dram_parameter("cbt", [4, 128, C], f32, isOutput=False)
    cbg = [
        nc.declare_dram_parameter(f"cbg{g}", [C, Dg], f32, isOutput=False)
        for g in range(G)
    ]
    quant = nc.declare_dram_parameter("quant", [T2, D], f32, isOutput=True)
    codes = nc.declare_dram_parameter("codes", [T2, G], u32, isOutput=True)

    with tile.TileContext(nc) as tc:
        with (
            tc.tile_pool(name="const", bufs=1) as constp,
            tc.tile_pool(name="stage1", bufs=1) as s1p,
            tc.tile_pool(name="cb", bufs=3) as cbp,
            tc.tile_pool(name="scores", bufs=3) as scp,
            tc.tile_pool(name="outs", bufs=1) as outp,
            tc.tile_pool(name="small", bufs=4) as smp,
            tc.tile_pool(name="probe", bufs=1) as probep,
            tc.tile_pool(name="psum2", bufs=2, space="PSUM") as ps2,
        ):
            ident = constp.tile([128, 128], f32)
            make_identity(nc, ident[:])

            # ---- Stage 1: stacked features (strided DMA) + transpose ----
            stT = [s1p.tile([KT, T2], f32, tag=f"stT{k}", name=f"stT{k}") for k in range(NKT)]
            stacked_t = []
            for rc, (r0, nr) in enumerate(ROW_CHUNKS):
                stacked = s1p.tile([128, KIN], f32, tag=f"stacked{rc}", name=f"stacked{rc}")
                src = bass.AP(
                    tensor=feats, offset=r0 * S * M, ap=[[S * M, nr], [1, KIN]]
                )
                nc.sync.dma_start(out=stacked[:nr, :], in_=src)
                stacked_t.append(stacked)
            for k in range(NKT):
                for rc, (r0, nr) in enumerate(ROW_CHUNKS):
                    pt = ps2.tile([KT, 128], f32, tag="ps", name="tp")
                    nc.tensor.transpose(
                        out=pt[:, :nr],
                        in_=stacked_t[rc][:nr, k * KT : (k + 1) * KT],
                        identity=ident[:nr, :nr],
                    )
                    nc.vector.tensor_copy(
                        out=stT[k][:, r0 : r0 + nr], in_=pt[:, :nr]
                    )

            # ---- projection weights ----
            projsb = []
            for k in range(NKT):
                pw = s1p.tile([KT, D], f32, tag=f"proj{k}", name=f"projsb{k}")
                nc.sync.dma_start(
                    out=pw[:], in_=proj[k * KT : (k + 1) * KT, :]
                )
                projsb.append(pw)

            # ---- xsT = (stacked @ proj).T  as 4 tiles [128, T2] ----
            xsT = [
                s1p.tile([128, T2], f32, tag=f"xsT{dc}", name=f"xsT{dc}")
                for dc in range(4)
            ]

            def emit_xsT_half(dc, gg):
                px = ps2.tile([64, T2], f32, tag="ps", name="pxh")
                for k in range(NKT):
                    nc.tensor.matmul(
                        px[:],
                        projsb[k][:, dc * 128 + gg * 64 : dc * 128 + gg * 64 + 64],
                        stT[k][:],
                        start=(k == 0),
                        stop=(k == NKT - 1),
                    )
                nc.vector.tensor_copy(
                    out=xsT[dc][gg * 64 : gg * 64 + 64, :], in_=px[:]
                )

            emit_xsT_half(0, 0)

            # ---- Stage 2: distances + argmax ----
            codes_sb = [
                outp.tile([128, G], u32, tag=f"codes{rc}", name=f"codes{rc}")
                for rc in range(len(ROW_CHUNKS))
            ]
            qsb_all = [
                outp.tile([128, D], f32, tag=f"qout{rc}", name=f"qout{rc}")
                for rc in range(len(ROW_CHUNKS))
            ]
            for q in range(4):  # group pair (2q, 2q+1)
                cbh = []
                for h in range(2):
                    cbt_h = cbp.tile([128, C // 2], f32, tag="cbt", name=f"cb{q}_{h}")
                    nc.sync.dma_start(
                        out=cbt_h[:], in_=cbt[q, :, h * (C // 2) : (h + 1) * (C // 2)]
                    )
                    cbh.append(cbt_h)
                # --- full row chunks: one scan unit per (rc, gg), sequential ---
                for rc, (r0, nr) in enumerate(ROW_CHUNKS[:2]):
                    for gg in range(2):
                        g = 2 * q + gg
                        if q == 0 and rc == 0 and gg == 1:
                            emit_xsT_half(0, 1)
                            for dcr in range(1, 4):
                                emit_xsT_half(dcr, 0)
                                emit_xsT_half(dcr, 1)
                        sc = scp.tile([128, C], f32, tag="scores", name="sc")
                        for half in range(4):
                            ps = ps2.tile([128, 2048], f32, tag="ps", name="ps")
                            for sub in range(4):
                                ct = half * 4 + sub
                                nc.tensor.matmul(
                                    ps[:nr, sub * 512 : (sub + 1) * 512],
                                    xsT[q][gg * 64 : gg * 64 + 64, r0 : r0 + nr],
                                    cbh[ct // 8][
                                        gg * 64 : gg * 64 + 64,
                                        (ct % 8) * 512 : (ct % 8 + 1) * 512,
                                    ],
                                    start=True,
                                    stop=True,
                                )
                            nc.scalar.copy(
                                out=sc[:nr, half * 2048 : (half + 1) * 2048],
                                in_=ps[:nr, :],
                            )
                        mx = smp.tile([128, 8], f32, tag="mx", name="mx")
                        ix = smp.tile([128, 8], u32, tag="ix", name="ix")
                        nc.vector.max(out=mx[:nr, :], in_=sc[:nr, :])
                        nc.vector.max_index(
                            out=ix[:nr, :], in_max=mx[:nr, :], in_values=sc[:nr, :]
                        )
                        nc.vector.tensor_copy(
                            out=codes_sb[rc][:nr, g : g + 1], in_=ix[:nr, 0:1]
                        )
                        nc.gpsimd.indirect_dma_start(
                            out=qsb_all[rc][:nr, g * Dg : (g + 1) * Dg],
                            out_offset=None,
                            in_=cbg[g][:],
                            in_offset=bass.IndirectOffsetOnAxis(
                                ap=ix[:nr, 0:1], axis=0
                            ),
                        )
                # --- packed 43-row chunk: both groups in one scan unit ---
                r0, nr = ROW_CHUNKS[2]
                scP = scp.tile([128, C], f32, tag="scores", name="scP")
                nc.gpsimd.memset(scP[32:64, :], -1.0e38)
                for half in range(4):
                    psp_t = ps2.tile([128, 2048], f32, tag="ps", name="psP")
                    for sub in range(4):
                        ct = half * 4 + sub
                        for gg in range(2):
                            nc.tensor.matmul(
                                psp_t[gg * 64 : gg * 64 + nr, sub * 512 : (sub + 1) * 512],
                                xsT[q][gg * 64 : gg * 64 + 64, r0 : r0 + nr],
                                cbh[ct // 8][
                                    gg * 64 : gg * 64 + 64,
                                    (ct % 8) * 512 : (ct % 8 + 1) * 512,
                                ],
                                start=True,
                                stop=True,
                                tile_position=(gg * 64, gg * 64),
                            )
                    nc.scalar.copy(
                        out=scP[:nr, half * 2048 : (half + 1) * 2048],
                        in_=psp_t[:nr, :],
                    )
                    nc.scalar.copy(
                        out=scP[64 : 64 + nr, half * 2048 : (half + 1) * 2048],
                        in_=psp_t[64 : 64 + nr, :],
                    )
                mxP = smp.tile([128, 8], f32, tag="mx", name="mxP")
                ixP = outp.tile([128, 8], u32, tag=f"ixP{q}", name=f"ixP{q}")
                nc.vector.max(out=mxP[: 64 + nr, :], in_=scP[: 64 + nr, :])
                nc.vector.max_index(
                    out=ixP[: 64 + nr, :], in_max=mxP[: 64 + nr, :], in_values=scP[: 64 + nr, :]
                )
                nc.vector.tensor_copy(
                    out=codes_sb[2][:nr, 2 * q : 2 * q + 1], in_=ixP[:nr, 0:1]
                )
                nc.sync.dma_start(
                    out=codes_sb[2][:nr, 2 * q + 1 : 2 * q + 2],
                    in_=ixP[64 : 64 + nr, 0:1],
                )
                for gg in range(2):
                    g = 2 * q + gg
                    nc.gpsimd.indirect_dma_start(
                        out=qsb_all[2][:nr, g * Dg : (g + 1) * Dg],
                        out_offset=None,
                        in_=cbg[g][:],
                        in_offset=bass.IndirectOffsetOnAxis(
                            ap=codes_sb[2][:nr, g : g + 1], axis=0
                        ),
                    )

            # ---- write outputs ----
            for rc, (r0, nr) in enumerate(ROW_CHUNKS):
                nc.sync.dma_start(out=quant[r0 : r0 + nr, :], in_=qsb_all[rc][:nr, :])
                nc.sync.dma_start(
                    out=codes[r0 : r0 + nr, :], in_=codes_sb[rc][:nr, :]
                )

    nc.compile()
    return nc


def _install_ntff_hook():
    """Shim antenv.axon_hooks so run_bass_kernel_spmd(trace=True) can profile."""
    import types

    try:
        from antenv.axon_hooks import get_axon_ntff_profile_hook  # noqa: F401

        return
    except ImportError:
        pass
    sys.path.insert(0, "/root/.axon_site")
    from trn_agent_boot.trn_boot import _ntff_profile_via_ctypes

    hook = _ntff_profile_via_ctypes("/opt/axon/libaxon_pjrt.so")
    mod = types.ModuleType("antenv.axon_hooks")
    mod.get_axon_ntff_profile_hook = lambda: hook
    mod.set_axon_ntff_profile_hook = lambda h: None
    import antenv

    sys.modules["antenv.axon_hooks"] = mod
    antenv.axon_hooks = mod


def kernel(feats, projection, codebook):
    from concourse.bass_utils import run_bass_kernel_spmd

    if os.environ.get("VQ_TRACE"):
        _install_ntff_hook()
    if "nc" not in _CACHE:
        _CACHE["nc"] = _build()
    nc = _CACHE["nc"]

    feats = np.ascontiguousarray(feats, dtype=np.float32)
    projection = np.ascontiguousarray(projection, dtype=np.float32)
    codebook = np.ascontiguousarray(codebook, dtype=np.float32)

    cbt_np = np.ascontiguousarray(
        codebook.transpose(1, 2, 0).reshape(4, 128, C)
    )
    cbg_np = [np.ascontiguousarray(codebook[:, g, :]) for g in range(G)]

    in_maps = []
    for b in range(NCORES):
        m = {
            "feats": np.ascontiguousarray(feats[b]),
            "proj": projection,
            "cbt": cbt_np,
        }
        for g in range(G):
            m[f"cbg{g}"] = cbg_np[g]
        in_maps.append(m)

    trace = bool(os.environ.get("VQ_TRACE"))
    res = run_bass_kernel_spmd(
        nc, in_maps, core_ids=list(range(NCORES)), trace=trace
    )
    _CACHE["profile"] = {
        "exec_time_ns": res.exec_time_ns,
        "instructions_and_trace": res.instructions_and_trace,
    }

    quant = np.stack([res.results[b]["quant"] for b in range(NCORES)])
    codes = np.stack([res.results[b]["codes"] for b in range(NCORES)])
    return quant.reshape(B, T2, D), codes.astype(np.int32).reshape(B, T2, G)


# revision 37
# speedup vs baseline: 1.0512x; 1.0512x over previous
"""BestRQ VQ kernel for 8 TRN2 NeuronCores.

Data-parallel over batch: core b handles feats[b] (299 stacked rows).
codes = argmax_c <x_g, cb_cg>  (L2-norm of x is a positive per-row scalar and
the codebook rows are unit-norm, so normalization does not change the argmin).
quantized = codebook[codes] gathered via indirect DMA.
"""

import os
import sys

import numpy as np

sys.path.insert(0, "/opt/trn_rl_repo")

B, T, M = 8, 1200, 80
F, S = 7, 4
T2 = (T - F) // S + 1  # 299
D, G, Dg, C = 512, 8, 64, 8192
KIN = F * M  # 560
KT = 112  # KIN split into 5 tiles of 112
NKT = 5
NCORES = 8
ROW_CHUNKS = [(0, 128), (128, 128), (256, T2 - 256)]  # (start, nrows)
NCT = C // 512  # 16 c-tiles of 512

_CACHE = {}


def _build():
    import concourse.bass as bass
    import concourse.tile as tile
    from concourse import bacc, mybir
    from concourse.masks import make_identity

    f32 = mybir.dt.float32
    f32r = mybir.dt.float32r
    u32 = mybir.dt.uint32

    nc = bacc.Bacc(None, target_bir_lowering=False)

    feats = nc.declare_dram_parameter("feats", [T, M], f32, isOutput=False)
    proj = nc.declare_dram_parameter("proj", [KIN, D], f32, isOutput=False)
    cbt = nc.declare_dram_parameter("cbt", [4, 128, C], f32, isOutput=False)
    cbg = [
        nc.declare_dram_parameter(f"cbg{g}", [C, Dg], f32, isOutput=False)
        for g in range(G)
    ]
    quant = nc.declare_dram_parameter("quant", [T2, D], f32, isOutput=True)
    codes = nc.declare_dram_parameter("codes", [T2, G], u32, isOutput=True)

    with tile.TileContext(nc) as tc:
        with (
            tc.tile_pool(name="const", bufs=1) as constp,
            tc.tile_pool(name="stage1", bufs=1) as s1p,
            tc.tile_pool(name="cb", bufs=3) as cbp,
            tc.tile_pool(name="scores", bufs=3) as scp,
            tc.tile_pool(name="outs", bufs=1) as outp,
            tc.tile_pool(name="small", bufs=4) as smp,
            tc.tile_pool(name="probe", bufs=1) as probep,
            tc.tile_pool(name="psum2", bufs=2, space="PSUM") as ps2,
        ):
            ident = constp.tile([128, 128], f32)
            make_identity(nc, ident[:])

            # HAM warm-up: ~10us of junk matmuls while input DMAs land, so
            # stage 1 and the first scan unit run at 2.4 GHz instead of 1.2.
            for w in range(24):
                jw = ps2.tile([64, 128], f32, tag="ps", name="warm")
                nc.tensor.matmul(
                    jw[:], ident[0:64, 0:64], ident[0:64, :], start=True, stop=True
                )

            # ---- Stage 1: stacked features (strided DMA) + transpose ----
            stT = [s1p.tile([KT, T2], f32, tag=f"stT{k}", name=f"stT{k}") for k in range(NKT)]
            for rc, (r0, nr) in enumerate(ROW_CHUNKS):
                stacked = s1p.tile([128, KIN], f32, tag=f"stacked{rc}", name=f"stacked{rc}")
                src = bass.AP(
                    tensor=feats, offset=r0 * S * M, ap=[[S * M, nr], [1, KIN]]
                )
                nc.sync.dma_start(out=stacked[:nr, :], in_=src)
                for k in range(NKT):
                    pt = ps2.tile([KT, 128], f32, tag="ps", name="tp")
                    nc.tensor.transpose(
                        out=pt[:, :nr],
                        in_=stacked[:nr, k * KT : (k + 1) * KT],
                        identity=ident[:nr, :nr],
                    )
                    nc.vector.tensor_copy(
                        out=stT[k][:, r0 : r0 + nr], in_=pt[:, :nr]
                    )

            # ---- projection weights ----
            projsb = []
            for k in range(NKT):
                pw = s1p.tile([KT, D], f32, tag=f"proj{k}", name=f"projsb{k}")
                nc.sync.dma_start(
                    out=pw[:], in_=proj[k * KT : (k + 1) * KT, :]
                )
                projsb.append(pw)

            # ---- xsT = (stacked @ proj).T  as 4 tiles [128, T2] ----
            xsT = []
            for dc in range(4):
                px = ps2.tile([128, T2], f32, tag="ps", name="px")
                for k in range(NKT):
                    nc.tensor.matmul(
                        px[:],
                        projsb[k][:, dc * 128 : (dc + 1) * 128],
                        stT[k][:],
                        start=(k == 0),
                        stop=(k == NKT - 1),
                    )
                xt = s1p.tile([128, T2], f32, tag=f"xsT{dc}", name=f"xsT{dc}")
                nc.vector.tensor_copy(out=xt[:], in_=px[:])
                xsT.append(xt)

            # ---- Stage 2: distances + argmax ----
            codes_sb = [
                outp.tile([128, G], u32, tag=f"codes{rc}", name=f"codes{rc}")
                for rc in range(len(ROW_CHUNKS))
            ]
            qsb_all = [
                outp.tile([128, D], f32, tag=f"qout{rc}", name=f"qout{rc}")
                for rc in range(len(ROW_CHUNKS))
            ]
            for q in range(4):  # group pair (2q, 2q+1)
                cbh = []
                for h in range(2):
                    cbt_h = cbp.tile([128, C // 2], f32, tag="cbt", name=f"cb{q}_{h}")
                    nc.sync.dma_start(
                        out=cbt_h[:], in_=cbt[q, :, h * (C // 2) : (h + 1) * (C // 2)]
                    )
                    cbh.append(cbt_h)
                # --- full row chunks: one scan unit per (rc, gg), sequential ---
                for rc, (r0, nr) in enumerate(ROW_CHUNKS[:2]):
                    for gg in range(2):
                        g = 2 * q + gg
                        sc = scp.tile([128, C], f32, tag="scores", name="sc")
                        for half in range(4):
                            ps = ps2.tile([128, 2048], f32, tag="ps", name="ps")
                            for sub in range(4):
                                ct = half * 4 + sub
                                nc.tensor.matmul(
                                    ps[:nr, sub * 512 : (sub + 1) * 512],
                                    xsT[q][gg * 64 : gg * 64 + 64, r0 : r0 + nr],
                                    cbh[ct // 8][
                                        gg * 64 : gg * 64 + 64,
                                        (ct % 8) * 512 : (ct % 8 + 1) * 512,
                                    ],
                                    start=True,
                                    stop=True,
                                )
                            nc.scalar.copy(
                                out=sc[:nr, half * 2048 : (half + 1) * 2048],
                                in_=ps[:nr, :],
                            )
                        mx = smp.tile([128, 8], f32, tag="mx", name="mx")
                        ix = smp.tile([128, 8], u32, tag="ix", name="ix")
                        nc.vector.max(out=mx[:nr, :], in_=sc[:nr, :])
                        nc.vector.max_index(
                            out=ix[:nr, :], in_max=mx[:nr, :], in_values=sc[:nr, :]
                        )
                        nc.vector.tensor_copy(
                            out=codes_sb[rc][:nr, g : g + 1], in_=ix[:nr, 0:1]
                        )
                        nc.gpsimd.indirect_dma_start(
                            out=qsb_all[rc][:nr, g * Dg : (g + 1) * Dg],
                            out_offset=None,
                            in_=cbg[g][:],
                            in_offset=bass.IndirectOffsetOnAxis(
                                ap=ix[:nr, 0:1], axis=0
                            ),
                        )
                # --- packed 43-row chunk: both groups in one scan unit ---
                r0, nr = ROW_CHUNKS[2]
                scP = scp.tile([128, C], f32, tag="scores", name="scP")
                nc.gpsimd.memset(scP[32:64, :], -1.0e38)
                for half in range(4):
                    psp_t = ps2.tile([128, 2048], f32, tag="ps", name="psP")
                    for sub in range(4):
                        ct = half * 4 + sub
                        for gg in range(2):
                            nc.tensor.matmul(
                                psp_t[gg * 64 : gg * 64 + nr, sub * 512 : (sub + 1) * 512],
                                xsT[q][gg * 64 : gg * 64 + 64, r0 : r0 + nr],
                                cbh[ct // 8][
                                    gg * 64 : gg * 64 + 64,
                                    (ct % 8) * 512 : (ct % 8 + 1) * 512,
                                ],
                                start=True,
                                stop=True,
                                tile_position=(gg * 64, gg * 64),
                            )
                    nc.scalar.copy(
                        out=scP[:nr, half * 2048 : (half + 1) * 2048],
                        in_=psp_t[:nr, :],
                    )
                    nc.scalar.copy(
                        out=scP[64 : 64 + nr, half * 2048 : (half + 1) * 2048],
                        in_=psp_t[64 : 64 + nr, :],
                    )
                mxP = smp.tile([128, 8], f32, tag="mx", name="mxP")
                ixP = outp.tile([128, 8], u32, tag=f"ixP{q}", name=f"ixP{q}")
                nc.vector.max(out=mxP[: 64 + nr, :], in_=scP[: 64 + nr, :])
                nc.vector.max_index(
                    out=ixP[: 64 + nr, :], in_max=mxP[: 64 + nr, :], in_values=scP[: 64 + nr, :]
                )
                nc.vector.tensor_copy(
                    out=codes_sb[2][:nr, 2 * q : 2 * q + 1], in_=ixP[:nr, 0:1]
                )
                nc.sync.dma_start(
                    out=codes_sb[2][:nr, 2 * q + 1 : 2 * q + 2],
                    in_=ixP[64 : 64 + nr, 0:1],
                )
                for gg in range(2):
                    g = 2 * q + gg
                    nc.gpsimd.indirect_dma_start(
                        out=qsb_all[2][:nr, g * Dg : (g + 1) * Dg],
                        out_offset=None,
                        in_=cbg[g][:],
                        in_offset=bass.IndirectOffsetOnAxis(
                            ap=codes_sb[2][:nr, g : g + 1], axis=0
                        ),
                    )

            # ---- write outputs ----
            for rc, (r0, nr) in enumerate(ROW_CHUNKS):
                nc.sync.dma_start(out=quant[r0 : r0 + nr, :], in_=qsb_all[rc][:nr, :])
                nc.sync.dma_start(
                    out=codes[r0 : r0 + nr, :], in_=codes_sb[rc][:nr, :]
                )

    nc.compile()
    return nc


def _install_ntff_hook():
    """Shim antenv.axon_hooks so run_bass_kernel_spmd(trace=True) can profile."""
    import types

    try:
        from antenv.axon_hooks import get_axon_ntff_profile_hook  # noqa: F401

        return
    except ImportError:
        pass
    sys.path.insert(0, "/root/.axon_site")
    from trn_agent_boot.trn_boot import _ntff_profile_via_ctypes

    hook = _ntff_profile_via_ctypes("/opt/axon/libaxon_pjrt.so")
    mod = types.ModuleType("antenv.axon_hooks")
    mod.get_axon_ntff_profile_hook = lambda: hook
    mod.set_axon_ntff_profile_hook = lambda h: None
    import antenv

    sys.modules["antenv.axon_hooks"] = mod
    antenv.axon_hooks = mod


def kernel(feats, projection, codebook):
    from concourse.bass_utils import run_bass_kernel_spmd

    if os.environ.get("VQ_TRACE"):
        _install_ntff_hook()
    if "nc" not in _CACHE:
        _CACHE["nc"] = _build()
    nc = _CACHE["nc"]

    feats = np.ascontiguousarray(feats, dtype=np.float32)
    projection = np.ascontiguousarray(projection, dtype=np.float32)
    codebook = np.ascontiguousarray(codebook, dtype=np.float32)

    cbt_np = np.ascontiguousarray(
        codebook.transpose(1, 2, 0).reshape(4, 128, C)
    )
    cbg_np = [np.ascontiguousarray(codebook[:, g, :]) for g in range(G)]

    in_maps = []
    for b in range(NCORES):
        m = {
            "feats": np.ascontiguousarray(feats[b]),
            "proj": projection,
            "cbt": cbt_np,
        }
        for g in range(G):
            m[f"cbg{g}"] = cbg_np[g]
        in_maps.append(m)

    trace = bool(os.environ.get("VQ_TRACE"))
    res = run_bass_kernel_spmd(
        nc, in_maps, core_ids=list(range(NCORES)), trace=trace
    )
    _CACHE["profile"] = {
        "exec_time_ns": res.exec_time_ns,
        "instructions_and_trace": res.instructions_and_trace,
    }

    quant = np.stack([res.results[b]["quant"] for b in range(NCORES)])
    codes = np.stack([res.results[b]["codes"] for b in range(NCORES)])
    return quant.reshape(B, T2, D), codes.astype(np.int32).reshape(B, T2, G)


# revision 38
# speedup vs baseline: 1.0622x; 1.0105x over previous
"""BestRQ VQ kernel for 8 TRN2 NeuronCores.

Data-parallel over batch: core b handles feats[b] (299 stacked rows).
codes = argmax_c <x_g, cb_cg>  (L2-norm of x is a positive per-row scalar and
the codebook rows are unit-norm, so normalization does not change the argmin).
quantized = codebook[codes] gathered via indirect DMA.
"""

import os
import sys

import numpy as np

sys.path.insert(0, "/opt/trn_rl_repo")

B, T, M = 8, 1200, 80
F, S = 7, 4
T2 = (T - F) // S + 1  # 299
D, G, Dg, C = 512, 8, 64, 8192
KIN = F * M  # 560
KT = 112  # KIN split into 5 tiles of 112
NKT = 5
NCORES = 8
ROW_CHUNKS = [(0, 128), (128, 128), (256, T2 - 256)]  # (start, nrows)
NCT = C // 512  # 16 c-tiles of 512

_CACHE = {}


def _build():
    import concourse.bass as bass
    import concourse.tile as tile
    from concourse import bacc, mybir
    from concourse.masks import make_identity

    f32 = mybir.dt.float32
    f32r = mybir.dt.float32r
    u32 = mybir.dt.uint32

    nc = bacc.Bacc(None, target_bir_lowering=False)

    feats = nc.declare_dram_parameter("feats", [T, M], f32, isOutput=False)
    proj = nc.declare_dram_parameter("proj", [KIN, D], f32, isOutput=False)
    cbt = nc.declare_dram_parameter("cbt", [4, 128, C], f32, isOutput=False)
    cbg = [
        nc.declare_dram_parameter(f"cbg{g}", [C, Dg], f32, isOutput=False)
        for g in range(G)
    ]
    quant = nc.declare_dram_parameter("quant", [T2, D], f32, isOutput=True)
    codes = nc.declare_dram_parameter("codes", [T2, G], u32, isOutput=True)

    with tile.TileContext(nc) as tc:
        with (
            tc.tile_pool(name="const", bufs=1) as constp,
            tc.tile_pool(name="stage1", bufs=1) as s1p,
            tc.tile_pool(name="cb", bufs=3) as cbp,
            tc.tile_pool(name="scores", bufs=3) as scp,
            tc.tile_pool(name="outs", bufs=1) as outp,
            tc.tile_pool(name="small", bufs=4) as smp,
            tc.tile_pool(name="probe", bufs=1) as probep,
            tc.tile_pool(name="psum2", bufs=2, space="PSUM") as ps2,
        ):
            ident = constp.tile([128, 128], f32)
            make_identity(nc, ident[:])

            # ---- Stage 1: stacked features (strided DMA) + transpose ----
            stT = [s1p.tile([KT, T2], f32, tag=f"stT{k}", name=f"stT{k}") for k in range(NKT)]
            for rc, (r0, nr) in enumerate(ROW_CHUNKS):
                stacked = s1p.tile([128, KIN], f32, tag=f"stacked{rc}", name=f"stacked{rc}")
                src = bass.AP(
                    tensor=feats, offset=r0 * S * M, ap=[[S * M, nr], [1, KIN]]
                )
                nc.sync.dma_start(out=stacked[:nr, :], in_=src)
                for k in range(NKT):
                    pt = ps2.tile([KT, 128], f32, tag="ps", name="tp")
                    nc.tensor.transpose(
                        out=pt[:, :nr],
                        in_=stacked[:nr, k * KT : (k + 1) * KT],
                        identity=ident[:nr, :nr],
                    )
                    nc.vector.tensor_copy(
                        out=stT[k][:, r0 : r0 + nr], in_=pt[:, :nr]
                    )

            # ---- projection weights ----
            projsb = []
            for k in range(NKT):
                pw = s1p.tile([KT, D], f32, tag=f"proj{k}", name=f"projsb{k}")
                nc.sync.dma_start(
                    out=pw[:], in_=proj[k * KT : (k + 1) * KT, :]
                )
                projsb.append(pw)

            # ---- xsT = (stacked @ proj).T  as 4 tiles [128, T2] ----
            xsT = []
            for dc in range(4):
                px = ps2.tile([128, T2], f32, tag="ps", name="px")
                for k in range(NKT):
                    nc.tensor.matmul(
                        px[:],
                        projsb[k][:, dc * 128 : (dc + 1) * 128],
                        stT[k][:],
                        start=(k == 0),
                        stop=(k == NKT - 1),
                    )
                xt = s1p.tile([128, T2], f32, tag=f"xsT{dc}", name=f"xsT{dc}")
                nc.vector.tensor_copy(out=xt[:], in_=px[:])
                xsT.append(xt)

            # ---- Stage 2: distances + argmax ----
            codes_sb = [
                outp.tile([128, G], u32, tag=f"codes{rc}", name=f"codes{rc}")
                for rc in range(len(ROW_CHUNKS))
            ]
            qsb_all = [
                outp.tile([128, D], f32, tag=f"qout{rc}", name=f"qout{rc}")
                for rc in range(len(ROW_CHUNKS))
            ]
            for q in range(4):  # group pair (2q, 2q+1)
                cbh = []
                for h in range(2):
                    cbt_h = cbp.tile([128, C // 2], f32, tag="cbt", name=f"cb{q}_{h}")
                    nc.sync.dma_start(
                        out=cbt_h[:], in_=cbt[q, :, h * (C // 2) : (h + 1) * (C // 2)]
                    )
                    cbh.append(cbt_h)
                # --- full row chunks: one scan unit per (rc, gg), sequential ---
                for rc, (r0, nr) in enumerate(ROW_CHUNKS[:2]):
                    for gg in range(2):
                        g = 2 * q + gg
                        sc = scp.tile([128, C], f32, tag="scores", name="sc")
                        for half in range(4):
                            ps = ps2.tile([128, 2048], f32, tag="ps", name="ps")
                            for sub in range(4):
                                ct = half * 4 + sub
                                nc.tensor.matmul(
                                    ps[:nr, sub * 512 : (sub + 1) * 512],
                                    xsT[q][gg * 64 : gg * 64 + 64, r0 : r0 + nr],
                                    cbh[ct // 8][
                                        gg * 64 : gg * 64 + 64,
                                        (ct % 8) * 512 : (ct % 8 + 1) * 512,
                                    ],
                                    start=True,
                                    stop=True,
                                )
                            nc.scalar.copy(
                                out=sc[:nr, half * 2048 : (half + 1) * 2048],
                                in_=ps[:nr, :],
                            )
                        mx = smp.tile([128, 8], f32, tag="mx", name="mx")
                        ix = smp.tile([128, 8], u32, tag="ix", name="ix")
                        nc.vector.max(out=mx[:nr, :], in_=sc[:nr, :])
                        nc.vector.max_index(
                            out=ix[:nr, :], in_max=mx[:nr, :], in_values=sc[:nr, :]
                        )
                        nc.vector.tensor_copy(
                            out=codes_sb[rc][:nr, g : g + 1], in_=ix[:nr, 0:1]
                        )
                        nc.gpsimd.indirect_dma_start(
                            out=qsb_all[rc][:nr, g * Dg : (g + 1) * Dg],
                            out_offset=None,
                            in_=cbg[g][:],
                            in_offset=bass.IndirectOffsetOnAxis(
                                ap=ix[:nr, 0:1], axis=0
                            ),
                        )
                # --- packed 43-row chunk: both groups in one scan unit ---
                r0, nr = ROW_CHUNKS[2]
                scP = scp.tile([128, C], f32, tag="scores", name="scP")
                nc.gpsimd.memset(scP[32:64, :], -1.0e38)
                for half in range(4):
                    psp_t = ps2.tile([128, 2048], f32, tag="ps", name="psP")
                    for sub in range(4):
                        ct = half * 4 + sub
                        for gg in range(2):
                            nc.tensor.matmul(
                                psp_t[gg * 64 : gg * 64 + nr, sub * 512 : (sub + 1) * 512],
                                xsT[q][gg * 64 : gg * 64 + 64, r0 : r0 + nr],
                                cbh[ct // 8][
                                    gg * 64 : gg * 64 + 64,
                                    (ct % 8) * 512 : (ct % 8 + 1) * 512,
                                ],
                                start=True,
                                stop=True,
                                tile_position=(gg * 64, gg * 64),
                            )
                    nc.scalar.copy(
                        out=scP[:nr, half * 2048 : (half + 1) * 2048],
                        in_=psp_t[:nr, :],
                    )
                    nc.scalar.copy(
                        out=scP[64 : 64 + nr, half * 2048 : (half + 1) * 2048],
                        in_=psp_t[64 : 64 + nr, :],
                    )
                mxP = smp.tile([128, 8], f32, tag="mx", name="mxP")
                ixP = outp.tile([128, 8], u32, tag=f"ixP{q}", name=f"ixP{q}")
                nc.vector.max(out=mxP[: 64 + nr, :], in_=scP[: 64 + nr, :])
                nc.vector.max_index(
                    out=ixP[: 64 + nr, :], in_max=mxP[: 64 + nr, :], in_values=scP[: 64 + nr, :]
                )
                nc.vector.tensor_copy(
                    out=codes_sb[2][:nr, 2 * q : 2 * q + 1], in_=ixP[:nr, 0:1]
                )
                nc.sync.dma_start(
                    out=codes_sb[2][:nr, 2 * q + 1 : 2 * q + 2],
                    in_=ixP[64 : 64 + nr, 0:1],
                )
                for gg in range(2):
                    g = 2 * q + gg
                    nc.gpsimd.indirect_dma_start(
                        out=qsb_all[2][:nr, g * Dg : (g + 1) * Dg],
                        out_offset=None,
                        in_=cbg[g][:],
                        in_offset=bass.IndirectOffsetOnAxis(
                            ap=codes_sb[2][:nr, g : g + 1], axis=0
                        ),
                    )

            # ---- write outputs ----
            for rc, (r0, nr) in enumerate(ROW_CHUNKS):
                nc.sync.dma_start(out=quant[r0 : r0 + nr, :], in_=qsb_all[rc][:nr, :])
                nc.sync.dma_start(
                    out=codes[r0 : r0 + nr, :], in_=codes_sb[rc][:nr, :]
                )

    nc.compile()
    return nc


def _install_ntff_hook():
    """Shim antenv.axon_hooks so run_bass_kernel_spmd(trace=True) can profile."""
    import types

    try:
        from antenv.axon_hooks import get_axon_ntff_profile_hook  # noqa: F401

        return
    except ImportError:
        pass
    sys.path.insert(0, "/root/.axon_site")
    from trn_agent_boot.trn_boot import _ntff_profile_via_ctypes

    hook = _ntff_profile_via_ctypes("/opt/axon/libaxon_pjrt.so")
    mod = types.ModuleType("antenv.axon_hooks")
    mod.get_axon_ntff_profile_hook = lambda: hook
    mod.set_axon_ntff_profile_hook = lambda h: None
    import antenv

    sys.modules["antenv.axon_hooks"] = mod
    antenv.axon_hooks = mod


def kernel(feats, projection, codebook):
    from concourse.bass_utils import run_bass_kernel_spmd

    if os.environ.get("VQ_TRACE"):
        _install_ntff_hook()
    if "nc" not in _CACHE:
        _CACHE["nc"] = _build()
    nc = _CACHE["nc"]

    feats = np.ascontiguousarray(feats, dtype=np.float32)
    projection = np.ascontiguousarray(projection, dtype=np.float32)
    codebook = np.ascontiguousarray(codebook, dtype=np.float32)

    cbt_np = np.ascontiguousarray(
        codebook.transpose(1, 2, 0).reshape(4, 128, C)
    )
    cbg_np = [np.ascontiguousarray(codebook[:, g, :]) for g in range(G)]

    in_maps = []
    for b in range(NCORES):
        m = {
            "feats": np.ascontiguousarray(feats[b]),
            "proj": projection,
            "cbt": cbt_np,
        }
        for g in range(G):
            m[f"cbg{g}"] = cbg_np[g]
        in_maps.append(m)

    trace = bool(os.environ.get("VQ_TRACE"))
    res = run_bass_kernel_spmd(
        nc, in_maps, core_ids=list(range(NCORES)), trace=trace
    )
    _CACHE["profile"] = {
        "exec_time_ns": res.exec_time_ns,
        "instructions_and_trace": res.instructions_and_trace,
    }

    quant = np.stack([res.results[b]["quant"] for b in range(NCORES)])
    codes = np.stack([res.results[b]["codes"] for b in range(NCORES)])
    return quant.reshape(B, T2, D), codes.astype(np.int32).reshape(B, T2, G)


# revision 46
# speedup vs baseline: 1.0874x; 1.0237x over previous
"""BestRQ VQ kernel for 8 TRN2 NeuronCores.

Data-parallel over batch: core b handles feats[b] (299 stacked rows).
codes = argmax_c <x_g, cb_cg>  (L2-norm of x is a positive per-row scalar and
the codebook rows are unit-norm, so normalization does not change the argmin).
quantized = codebook[codes] gathered via indirect DMA.
"""

import os
import sys

import numpy as np

sys.path.insert(0, "/opt/trn_rl_repo")

B, T, M = 8, 1200, 80
F, S = 7, 4
T2 = (T - F) // S + 1  # 299
D, G, Dg, C = 512, 8, 64, 8192
KIN = F * M  # 560
KT = 112  # KIN split into 5 tiles of 112
NKT = 5
NCORES = 8
ROW_CHUNKS = [(0, 128), (128, 128), (256, T2 - 256)]  # (start, nrows)
NCT = C // 512  # 16 c-tiles of 512

_CACHE = {}


def _build():
    import concourse.bass as bass
    import concourse.tile as tile
    from concourse import bacc, mybir
    from concourse.masks import make_identity

    f32 = mybir.dt.float32
    f32r = mybir.dt.float32r
    u32 = mybir.dt.uint32

    nc = bacc.Bacc(None, target_bir_lowering=False)

    feats = nc.declare_dram_parameter("feats", [T, M], f32, isOutput=False)
    proj = nc.declare_dram_parameter("proj", [KIN, D], f32, isOutput=False)
    cbt = nc.declare_dram_parameter("cbt", [4, 128, C], f32, isOutput=False)
    cbg = [
        nc.declare_dram_parameter(f"cbg{g}", [C, Dg], f32, isOutput=False)
        for g in range(G)
    ]
    quant = nc.declare_dram_parameter("quant", [T2, D], f32, isOutput=True)
    codes = nc.declare_dram_parameter("codes", [T2, G], u32, isOutput=True)

    with tile.TileContext(nc) as tc:
        with (
            tc.tile_pool(name="const", bufs=1) as constp,
            tc.tile_pool(name="stage1", bufs=1) as s1p,
            tc.tile_pool(name="cb", bufs=3) as cbp,
            tc.tile_pool(name="scores", bufs=3) as scp,
            tc.tile_pool(name="outs", bufs=1) as outp,
            tc.tile_pool(name="small", bufs=4) as smp,
            tc.tile_pool(name="probe", bufs=1) as probep,
            tc.tile_pool(name="psum2", bufs=2, space="PSUM") as ps2,
        ):
            ident = constp.tile([128, 128], f32)
            make_identity(nc, ident[:])

            # ---- Stage 1: stacked features (strided DMA) + transpose ----
            stT = [s1p.tile([KT, T2], f32, tag=f"stT{k}", name=f"stT{k}") for k in range(NKT)]
            for rc, (r0, nr) in enumerate(ROW_CHUNKS):
                stacked = s1p.tile([128, KIN], f32, tag=f"stacked{rc}", name=f"stacked{rc}")
                src = bass.AP(
                    tensor=feats, offset=r0 * S * M, ap=[[S * M, nr], [1, KIN]]
                )
                nc.sync.dma_start(out=stacked[:nr, :], in_=src)
                for k in range(NKT):
                    pt = ps2.tile([KT, 128], f32, tag="ps", name="tp")
                    nc.tensor.transpose(
                        out=pt[:, :nr],
                        in_=stacked[:nr, k * KT : (k + 1) * KT],
                        identity=ident[:nr, :nr],
                    )
                    nc.vector.tensor_copy(
                        out=stT[k][:, r0 : r0 + nr], in_=pt[:, :nr]
                    )

            # ---- projection weights ----
            projsb = []
            for k in range(NKT):
                pw = s1p.tile([KT, D], f32, tag=f"proj{k}", name=f"projsb{k}")
                nc.sync.dma_start(
                    out=pw[:], in_=proj[k * KT : (k + 1) * KT, :]
                )
                projsb.append(pw)

            # ---- xsT = (stacked @ proj).T  as 4 tiles [128, T2] ----
            xsT = []
            for dc in range(4):
                px = ps2.tile([128, T2], f32, tag="ps", name="px")
                for k in range(NKT):
                    nc.tensor.matmul(
                        px[:],
                        projsb[k][:, dc * 128 : (dc + 1) * 128],
                        stT[k][:],
                        start=(k == 0),
                        stop=(k == NKT - 1),
                    )
                xt = s1p.tile([128, T2], f32, tag=f"xsT{dc}", name=f"xsT{dc}")
                nc.vector.tensor_copy(out=xt[:], in_=px[:])
                xsT.append(xt)

            # ---- Stage 2: distances + argmax ----
            codes_sb = [
                outp.tile([128, G], u32, tag=f"codes{rc}", name=f"codes{rc}")
                for rc in range(len(ROW_CHUNKS))
            ]
            qsb_all = [
                outp.tile([128, D], f32, tag=f"qout{rc}", name=f"qout{rc}")
                for rc in range(len(ROW_CHUNKS))
            ]
            for q in range(4):  # group pair (2q, 2q+1)
                cbh = []
                for h in range(2):
                    cbt_h = cbp.tile([128, C // 2], f32, tag="cbt", name=f"cb{q}_{h}")
                    nc.sync.dma_start(
                        out=cbt_h[:], in_=cbt[q, :, h * (C // 2) : (h + 1) * (C // 2)]
                    )
                    cbh.append(cbt_h)
                def packed_unit():
                    # --- packed 43-row chunk: both groups in one scan unit ---
                    r0, nr = ROW_CHUNKS[2]
                    scP = scp.tile([128, C], f32, tag="scores", name="scP")
                    nc.gpsimd.memset(scP[32:64, :], -1.0e38)
                    for half in range(4):
                        psp_t = ps2.tile([128, 2048], f32, tag="ps", name="psP")
                        for sub in range(4):
                            ct = half * 4 + sub
                            for gg in range(2):
                                nc.tensor.matmul(
                                    psp_t[gg * 64 : gg * 64 + nr, sub * 512 : (sub + 1) * 512],
                                    xsT[q][gg * 64 : gg * 64 + 64, r0 : r0 + nr],
                                    cbh[ct // 8][
                                        gg * 64 : gg * 64 + 64,
                                        (ct % 8) * 512 : (ct % 8 + 1) * 512,
                                    ],
                                    start=True,
                                    stop=True,
                                    tile_position=(gg * 64, gg * 64),
                                )
                        nc.scalar.copy(
                            out=scP[:nr, half * 2048 : (half + 1) * 2048],
                            in_=psp_t[:nr, :],
                        )
                        nc.scalar.copy(
                            out=scP[64 : 64 + nr, half * 2048 : (half + 1) * 2048],
                            in_=psp_t[64 : 64 + nr, :],
                        )
                    mxP = smp.tile([128, 8], f32, tag="mx", name="mxP")
                    ixP = outp.tile([128, 8], u32, tag=f"ixP{q}", name=f"ixP{q}")
                    nc.vector.max(out=mxP[: 64 + nr, :], in_=scP[: 64 + nr, :])
                    nc.vector.max_index(
                        out=ixP[: 64 + nr, :], in_max=mxP[: 64 + nr, :], in_values=scP[: 64 + nr, :]
                    )
                    nc.vector.tensor_copy(
                        out=codes_sb[2][:nr, 2 * q : 2 * q + 1], in_=ixP[:nr, 0:1]
                    )
                    nc.sync.dma_start(
                        out=codes_sb[2][:nr, 2 * q + 1 : 2 * q + 2],
                        in_=ixP[64 : 64 + nr, 0:1],
                    )
                    for gg in range(2):
                        g = 2 * q + gg
                        nc.gpsimd.indirect_dma_start(
                            out=qsb_all[2][:nr, g * Dg : (g + 1) * Dg],
                            out_offset=None,
                            in_=cbg[g][:],
                            in_offset=bass.IndirectOffsetOnAxis(
                                ap=codes_sb[2][:nr, g : g + 1], axis=0
                            ),
                        )


                if q == 3:
                    packed_unit()
                # --- full row chunks: one scan unit per (rc, gg), sequential ---
                for rc, (r0, nr) in enumerate(ROW_CHUNKS[:2]):
                    for gg in range(2):
                        g = 2 * q + gg
                        first_unit = q == 0 and rc == 0 and gg == 0
                        sc = scp.tile([128, C], f32, tag="scores", name="sc")
                        mxA = ixA = None
                        for half in range(4):
                            ps = ps2.tile([128, 2048], f32, tag="ps", name="ps")
                            for sub in range(4):
                                ct = half * 4 + sub
                                nc.tensor.matmul(
                                    ps[:nr, sub * 512 : (sub + 1) * 512],
                                    xsT[q][gg * 64 : gg * 64 + 64, r0 : r0 + nr],
                                    cbh[ct // 8][
                                        gg * 64 : gg * 64 + 64,
                                        (ct % 8) * 512 : (ct % 8 + 1) * 512,
                                    ],
                                    start=True,
                                    stop=True,
                                )
                            nc.scalar.copy(
                                out=sc[:nr, half * 2048 : (half + 1) * 2048],
                                in_=ps[:nr, :],
                            )
                            if first_unit and half == 1:
                                # scan the first 4096 while the rest compute
                                mxA = smp.tile([128, 8], f32, tag="mxA", name="mxA")
                                ixA = smp.tile([128, 8], u32, tag="ixA", name="ixA")
                                nc.vector.max(out=mxA[:nr, :], in_=sc[:nr, : C // 2])
                                nc.vector.max_index(
                                    out=ixA[:nr, :], in_max=mxA[:nr, :],
                                    in_values=sc[:nr, : C // 2],
                                )
                        mx = smp.tile([128, 8], f32, tag="mx", name="mx")
                        ix = smp.tile([128, 8], u32, tag="ix", name="ix")
                        if first_unit:
                            nc.vector.max(out=mx[:nr, :], in_=sc[:nr, C // 2 :])
                            nc.vector.max_index(
                                out=ix[:nr, :], in_max=mx[:nr, :],
                                in_values=sc[:nr, C // 2 :],
                            )
                            fA = smp.tile([128, 1], f32, tag="fA", name="fA")
                            fB = smp.tile([128, 1], f32, tag="fB", name="fB")
                            bet = smp.tile([128, 1], f32, tag="bet", name="bet")
                            cdf = smp.tile([128, 1], f32, tag="cdf", name="cdf")
                            cdu = smp.tile([128, 1], u32, tag="cdu", name="cdu")
                            nc.vector.tensor_copy(out=fA[:nr, :], in_=ixA[:nr, 0:1])
                            nc.vector.tensor_copy(out=fB[:nr, :], in_=ix[:nr, 0:1])
                            nc.vector.tensor_scalar(
                                fB[:nr, :], fB[:nr, :], float(C // 2),
                                scalar2=None, op0=mybir.AluOpType.add,
                            )
                            nc.vector.tensor_tensor(
                                out=bet[:nr, :], in0=mxA[:nr, 0:1],
                                in1=mx[:nr, 0:1], op=mybir.AluOpType.is_ge,
                            )
                            dAB = smp.tile([128, 1], f32, tag="dAB", name="dAB")
                            nc.vector.tensor_tensor(
                                out=dAB[:nr, :], in0=fA[:nr, :], in1=fB[:nr, :],
                                op=mybir.AluOpType.subtract,
                            )
                            nc.vector.scalar_tensor_tensor(
                                out=cdf[:nr, :], in0=bet[:nr, :],
                                scalar=dAB[:nr, :], in1=fB[:nr, :],
                                op0=mybir.AluOpType.mult, op1=mybir.AluOpType.add,
                            )
                            nc.vector.tensor_copy(out=cdu[:nr, :], in_=cdf[:nr, :])
                            code_ap = cdu
                        else:
                            nc.vector.max(out=mx[:nr, :], in_=sc[:nr, :])
                            nc.vector.max_index(
                                out=ix[:nr, :], in_max=mx[:nr, :], in_values=sc[:nr, :]
                            )
                            code_ap = ix
                        nc.vector.tensor_copy(
                            out=codes_sb[rc][:nr, g : g + 1], in_=code_ap[:nr, 0:1]
                        )
                        nc.gpsimd.indirect_dma_start(
                            out=qsb_all[rc][:nr, g * Dg : (g + 1) * Dg],
                            out_offset=None,
                            in_=cbg[g][:],
                            in_offset=bass.IndirectOffsetOnAxis(
                                ap=code_ap[:nr, 0:1], axis=0
                            ),
                        )
                if q != 3:
                    packed_unit()
            # ---- write outputs ----
            for rc, (r0, nr) in enumerate(ROW_CHUNKS):
                nc.sync.dma_start(out=quant[r0 : r0 + nr, :], in_=qsb_all[rc][:nr, :])
                nc.sync.dma_start(
                    out=codes[r0 : r0 + nr, :], in_=codes_sb[rc][:nr, :]
                )

    nc.compile()
    return nc


def _install_ntff_hook():
    """Shim antenv.axon_hooks so run_bass_kernel_spmd(trace=True) can profile."""
    import types

    try:
        from antenv.axon_hooks import get_axon_ntff_profile_hook  # noqa: F401

        return
    except ImportError:
        pass
    sys.path.insert(0, "/root/.axon_site")
    from trn_agent_boot.trn_boot import _ntff_profile_via_ctypes

    hook = _ntff_profile_via_ctypes("/opt/axon/libaxon_pjrt.so")
    mod = types.ModuleType("antenv.axon_hooks")
    mod.get_axon_ntff_profile_hook = lambda: hook
    mod.set_axon_ntff_profile_hook = lambda h: None
    import antenv

    sys.modules["antenv.axon_hooks"] = mod
    antenv.axon_hooks = mod


def kernel(feats, projection, codebook):
    from concourse.bass_utils import run_bass_kernel_spmd

    if os.environ.get("VQ_TRACE"):
        _install_ntff_hook()
    if "nc" not in _CACHE:
        _CACHE["nc"] = _build()
    nc = _CACHE["nc"]

    feats = np.ascontiguousarray(feats, dtype=np.float32)
    projection = np.ascontiguousarray(projection, dtype=np.float32)
    codebook = np.ascontiguousarray(codebook, dtype=np.float32)

    cbt_np = np.ascontiguousarray(
        codebook.transpose(1, 2, 0).reshape(4, 128, C)
    )
    cbg_np = [np.ascontiguousarray(codebook[:, g, :]) for g in range(G)]

    in_maps = []
    for b in range(NCORES):
        m = {
            "feats": np.ascontiguousarray(feats[b]),
            "proj": projection,
            "cbt": cbt_np,
        }
        for g in range(G):
            m[f"cbg{g}"] = cbg_np[g]
        in_maps.append(m)

    trace = bool(os.environ.get("VQ_TRACE"))
    res = run_bass_kernel_spmd(
        nc, in_maps, core_ids=list(range(NCORES)), trace=trace
    )
    _CACHE["profile"] = {
        "exec_time_ns": res.exec_time_ns,
        "instructions_and_trace": res.instructions_and_trace,
    }

    quant = np.stack([res.results[b]["quant"] for b in range(NCORES)])
    codes = np.stack([res.results[b]["codes"] for b in range(NCORES)])
    return quant.reshape(B, T2, D), codes.astype(np.int32).reshape(B, T2, G)
